# revision 1
# baseline (speedup 1.0000x reference)
"""Trainium2 Bass kernel for CornerBoundingBoxEMDLoss.

For each sample: 8x8 pairwise corner distances, then exact min-cost perfect
matching. Instead of brute-forcing all 8! = 40320 permutations (the reference
does a [B,64]@[64,40320] GEMM + row-min), we use meet-in-the-middle:

  min over perms = min over 70 4-subsets T of
      (min assignment of preds {0,1,2,3} onto T)
    + (min assignment of preds {4,5,6,7} onto complement(T))

computed hierarchically: pred pairs -> target pairs (L1, one-hot GEMM with
two orderings + elementwise min), pairs -> quads (L2, one-hot GEMM over
2+2 splits + group-min-of-6), then the complement-aligned A+B pairing with a
fused add+min reduction (L3). Exact same minimum, ~50x less arithmetic.

Data-parallel across 8 NeuronCores: 512 samples per core, processed as
4 chunks of 128 samples (samples on SBUF partitions, transposed to
coord-major via PE transpose for the selection GEMMs).
"""

import itertools

import numpy as np

import concourse.bacc as bacc
import concourse.mybir as mybir
import concourse.tile as tile

N_CORES = 8
B_TOTAL = 4096
B_CORE = B_TOTAL // N_CORES          # 512
N_CHUNKS = 4
CHUNK = B_CORE // N_CHUNKS           # 128

F32 = mybir.dt.float32
# dtype used for the one-hot selection GEMMs (fp32 exact; float32r is 4x
# faster on the PE and exact for 0/1 weights if its decomposition holds --
# verified empirically before enabling).
GEMM_DT = mybir.dt.float32

MIN_INIT = 1.0e30


def _build_constants():
    """One-hot selection matrices for the two GEMM levels."""
    pairs = list(itertools.combinations(range(8), 2))            # 28
    pair_idx = {p: i for i, p in enumerate(pairs)}
    subs4 = list(itertools.combinations(range(8), 4))            # 70
    pred_pairs = [(0, 1), (2, 3), (4, 5), (6, 7)]

    l1o0 = np.zeros((64, 112), dtype=np.float32)
    l1o1 = np.zeros((64, 112), dtype=np.float32)
    for q, (i0, i1) in enumerate(pred_pairs):
        for p, (a, b) in enumerate(pairs):
            col = q * 28 + p
            l1o0[i0 * 8 + a, col] = 1; l1o0[i1 * 8 + b, col] = 1
            l1o1[i0 * 8 + b, col] = 1; l1o1[i1 * 8 + a, col] = 1

    l2 = np.zeros((112, 840), dtype=np.float32)
    for t, T in enumerate(subs4):
        for s, S in enumerate(itertools.combinations(T, 2)):
            rest = tuple(sorted(set(T) - set(S)))
            l2[0 * 28 + pair_idx[S], t * 6 + s] = 1
            l2[1 * 28 + pair_idx[rest], t * 6 + s] = 1
        TB = tuple(sorted(set(range(8)) - set(T)))               # complement
        for s, S in enumerate(itertools.combinations(TB, 2)):
            rest = tuple(sorted(set(TB) - set(S)))
            l2[2 * 28 + pair_idx[S], 420 + t * 6 + s] = 1
            l2[3 * 28 + pair_idx[rest], 420 + t * 6 + s] = 1

    ident = np.eye(128, dtype=np.float32)
    return l1o0, l1o1, l2, ident


def build_nc():
    nc = bacc.Bacc("TRN2", target_bir_lowering=False, debug=False)

    pred_d = nc.dram_tensor("pred", [B_CORE, 24], F32, kind="ExternalInput")
    targn_d = nc.dram_tensor("targn", [B_CORE, 24], F32, kind="ExternalInput")
    l1o0_d = nc.dram_tensor("l1o0", [64, 112], GEMM_DT, kind="ExternalInput")
    l1o1_d = nc.dram_tensor("l1o1", [64, 112], GEMM_DT, kind="ExternalInput")
    l2_d = nc.dram_tensor("l2mat", [112, 840], GEMM_DT, kind="ExternalInput")
    id_d = nc.dram_tensor("ident", [128, 128], F32, kind="ExternalInput")
    out_d = nc.dram_tensor("out", [B_CORE], F32, kind="ExternalOutput")

    with tile.TileContext(nc) as tc:
        with (
            tc.tile_pool(name="consts", bufs=1) as cpool,
            tc.tile_pool(name="persist", bufs=1) as ppool,
            tc.tile_pool(name="work", bufs=3) as wpool,
            tc.tile_pool(name="psum_t", bufs=2, space="PSUM") as pst,
            tc.tile_pool(name="psum_l1", bufs=1, space="PSUM") as psl1,
            tc.tile_pool(name="psum_l2", bufs=2, space="PSUM") as psl2,
        ):
            c_l1o0 = cpool.tile([64, 112], GEMM_DT, tag="l1o0")
            c_l1o1 = cpool.tile([64, 112], GEMM_DT, tag="l1o1")
            c_l2 = cpool.tile([112, 840], GEMM_DT, tag="l2")
            c_id = cpool.tile([128, 128], F32, tag="ident")
            nc.sync.dma_start(c_l1o0[:, :], l1o0_d[:, :])
            nc.sync.dma_start(c_l1o1[:, :], l1o1_d[:, :])
            nc.sync.dma_start(c_l2[:, :], l2_d[:, :])
            nc.sync.dma_start(c_id[:, :], id_d[:, :])

            distT = ppool.tile([64, B_CORE], GEMM_DT, tag="distT")
            m_t = ppool.tile([112, B_CORE], GEMM_DT, tag="m")
            loss = ppool.tile([128, N_CHUNKS], F32, tag="loss")

            # ---- phase 1: pairwise distances, transposed to [64, 512] ----
            for c in range(N_CHUNKS):
                sl = slice(c * CHUNK, (c + 1) * CHUNK)
                p_t = wpool.tile([128, 24], F32, tag="p")
                t_t = wpool.tile([128, 24], F32, tag="t")
                nc.sync.dma_start(p_t[:, :], pred_d[sl, :])
                nc.sync.dma_start(t_t[:, :], targn_d[sl, :])

                # diff[b, i, j, c3] = pred[b, i, c3] + (-target[b, j, c3])
                diff = wpool.tile([128, 192], F32, tag="diff")
                p_b = (p_t[:, :].rearrange("p (i c) -> p i c", i=8)
                       .unsqueeze(2).broadcast_to((128, 8, 8, 3)))
                t_b = (t_t[:, :].rearrange("p (j c) -> p j c", j=8)
                       .unsqueeze(1).broadcast_to((128, 8, 8, 3)))
                d4 = diff[:, :].rearrange("p (i j c) -> p i j c", i=8, j=8)
                nc.gpsimd.tensor_add(d4, p_b, t_b)

                sq = wpool.tile([128, 192], F32, tag="sq")
                nc.scalar.activation(sq[:, :], diff[:, :],
                                     mybir.ActivationFunctionType.Square)

                d2 = wpool.tile([128, 64], F32, tag="d2")
                nc.vector.tensor_reduce(
                    d2[:, :], sq[:, :].rearrange("p (r c) -> p r c", c=3),
                    axis=mybir.AxisListType.X, op=mybir.AluOpType.add)

                tp = pst.tile([64, 128], F32, tag="tp")
                nc.tensor.transpose(tp[:, :], d2[:, :], c_id[:, :])

                # sqrt fused with the PSUM->SBUF copy
                nc.scalar.activation(distT[:, sl], tp[:, :],
                                     mybir.ActivationFunctionType.Sqrt)

            # ---- L1: pred-pair x target-pair costs, both orderings ----
            ps0 = psl1.tile([112, B_CORE], F32, tag="ps0")
            ps1 = psl1.tile([112, B_CORE], F32, tag="ps1")
            nc.tensor.matmul(ps0[:, :], c_l1o0[:, :], distT[:, :],
                             start=True, stop=True)
            nc.tensor.matmul(ps1[:, :], c_l1o1[:, :], distT[:, :],
                             start=True, stop=True)
            # HW: TensorTensor may read at most one input from PSUM
            s1 = ppool.tile([112, B_CORE], F32, tag="s1")
            nc.scalar.activation(s1[:, :], ps1[:, :],
                                 mybir.ActivationFunctionType.Copy)
            nc.vector.tensor_tensor(m_t[:, :], ps0[:, :], s1[:, :],
                                    op=mybir.AluOpType.min)

            # ---- L2 + L3 per chunk ----
            for c in range(N_CHUNKS):
                sl = slice(c * CHUNK, (c + 1) * CHUNK)
                ps2 = psl2.tile([128, 1024], F32, tag="ps2")
                nc.tensor.matmul(ps2[:, 0:420], m_t[:, sl], c_l2[:, 0:420],
                                 start=True, stop=True)
                nc.tensor.matmul(ps2[:, 512:932], m_t[:, sl], c_l2[:, 420:840],
                                 start=True, stop=True)

                minab = wpool.tile([128, 140], F32, tag="minab")
                v = (ps2[:, :].rearrange("p (h x) -> p h x", h=2)[:, :, 0:420]
                     .rearrange("p h (t s) -> p h t s", s=6))
                nc.vector.tensor_reduce(minab[:, :], v,
                                        axis=mybir.AxisListType.X,
                                        op=mybir.AluOpType.min)

                scratch = wpool.tile([128, 70], F32, tag="scratch")
                nc.vector.tensor_tensor(scratch[:, :], minab[:, 0:70],
                                        minab[:, 70:140],
                                        op=mybir.AluOpType.add)
                nc.vector.tensor_reduce(loss[:, c:c + 1], scratch[:, :],
                                        axis=mybir.AxisListType.X,
                                        op=mybir.AluOpType.min)

            # loss[p, c] -> out[c*128 + p]
            nc.sync.dma_start(
                out_d[:].rearrange("(c p) -> p c", p=128), loss[:, :])

    nc.compile()
    return nc


_NC = None


def _get_nc():
    global _NC
    if _NC is None:
        _NC = build_nc()
    return _NC


def kernel(pred_corners: np.ndarray, target_corners: np.ndarray) -> np.ndarray:
    from concourse.bass_utils import run_bass_kernel_spmd

    nc = _get_nc()
    l1o0, l1o1, l2, ident = _build_constants()
    pred = np.ascontiguousarray(pred_corners, dtype=np.float32).reshape(B_TOTAL, 24)
    targn = -np.ascontiguousarray(target_corners, dtype=np.float32).reshape(B_TOTAL, 24)

    in_maps = []
    for k in range(N_CORES):
        sl = slice(k * B_CORE, (k + 1) * B_CORE)
        in_maps.append({
            "pred": pred[sl], "targn": targn[sl],
            "l1o0": l1o0, "l1o1": l1o1, "l2mat": l2, "ident": ident,
        })

    res = run_bass_kernel_spmd(nc, in_maps, core_ids=list(range(N_CORES)))
    return np.concatenate([res.results[k]["out"] for k in range(N_CORES)])



# revision 5
# speedup vs baseline: 1.9975x; 1.9975x over previous
"""Trainium2 Bass kernel for CornerBoundingBoxEMDLoss.

For each sample: 8x8 pairwise corner distances, then exact min-cost perfect
matching via meet-in-the-middle (identical math to the brute-force 8! GEMM):

  min over perms = min over 70 4-subsets T of
      (min assignment of preds {0,1,2,3} onto T)
    + (min assignment of preds {4,5,6,7} onto complement(T))

Pipeline (all GEMMs fp16, one packed input DMA, one contiguous output DMA):
  phase1: diff = one-hot selection GEMM over [predT; -targT]   (PE)
          square (Act), coord-sum GEMM (PE), sqrt -> distT fp16 (Act)
  L1:     pred-pair x target-pair costs, both orderings, in one 2-bank
          PSUM tile; fused cross-bank min reduce (DVE) -> m_t [112,512]
  L2:     quad costs via one-hot GEMM [112 -> 840] per 128-sample chunk
  L3:     min-of-6 reduce, then fused add+min70 (tensor_tensor_reduce)
  out:    PE-transpose loss [128,4] -> [4,128], one 2KB DMA

Data-parallel across 8 NeuronCores: 512 samples per core.
"""

import itertools

import numpy as np

import concourse.bacc as bacc
import concourse.mybir as mybir
import concourse.tile as tile

N_CORES = 8
B_TOTAL = 4096
B_CORE = B_TOTAL // N_CORES          # 512
N_CHUNKS = 4
CHUNK = B_CORE // N_CHUNKS           # 128

F32 = mybir.dt.float32
F16 = mybir.dt.float16

MIN_INIT = 1.0e30
N_WARMUP = 6                          # PE warmup matmuls during DMA wait

# packed input buffer column layout (fp16, [128, W])
C_DATA = 0        # [48, 512]  predT rows 0:24, -targT rows 24:48
C_A1 = 512        # [48, 128]  diff selection, coords c in {0,1}
C_A2 = 640        # [48, 64]   diff selection, coord c = 2
C_CS1 = 704       # [128, 64]  coord-sum for sq1
C_CS2 = 768       # [64, 64]   coord-sum for sq2 (identity)
C_L1 = 832        # [64, 224]  pair tables, both orderings
C_L2 = 1056       # [112, 840] quad tables (A cols 0:420, B cols 420:840)
W_IN = 1896


def _build_tables():
    """Host-side constant tables, all fp16-exact (0/1 entries)."""
    a1 = np.zeros((48, 128), dtype=np.float16)
    a2 = np.zeros((48, 64), dtype=np.float16)
    cs1 = np.zeros((128, 64), dtype=np.float16)
    cs2 = np.eye(64, dtype=np.float16)
    for i in range(8):
        for j in range(8):
            q = i * 8 + j
            for c in (0, 1):
                a1[i * 3 + c, c * 64 + q] = 1
                a1[24 + j * 3 + c, c * 64 + q] = 1
                cs1[c * 64 + q, q] = 1
            a2[i * 3 + 2, q] = 1
            a2[24 + j * 3 + 2, q] = 1

    pairs = list(itertools.combinations(range(8), 2))            # 28
    pair_idx = {p: i for i, p in enumerate(pairs)}
    subs4 = list(itertools.combinations(range(8), 4))            # 70
    pred_pairs = [(0, 1), (2, 3), (4, 5), (6, 7)]

    l1t = np.zeros((64, 224), dtype=np.float16)
    for q, (i0, i1) in enumerate(pred_pairs):
        for p, (a, b) in enumerate(pairs):
            col = q * 28 + p
            l1t[i0 * 8 + a, col] = 1
            l1t[i1 * 8 + b, col] = 1
            l1t[i0 * 8 + b, 112 + col] = 1
            l1t[i1 * 8 + a, 112 + col] = 1

    l2t = np.zeros((112, 840), dtype=np.float16)
    for t, T in enumerate(subs4):
        for s, S in enumerate(itertools.combinations(T, 2)):
            rest = tuple(sorted(set(T) - set(S)))
            l2t[0 * 28 + pair_idx[S], t * 6 + s] = 1
            l2t[1 * 28 + pair_idx[rest], t * 6 + s] = 1
        TB = tuple(sorted(set(range(8)) - set(T)))               # complement
        for s, S in enumerate(itertools.combinations(TB, 2)):
            rest = tuple(sorted(set(TB) - set(S)))
            l2t[2 * 28 + pair_idx[S], 420 + t * 6 + s] = 1
            l2t[3 * 28 + pair_idx[rest], 420 + t * 6 + s] = 1

    return a1, a2, cs1, cs2, l1t, l2t


def make_in_maps(pred_corners: np.ndarray, target_corners: np.ndarray):
    """Pack per-core [128, W_IN] fp16 input buffers."""
    a1, a2, cs1, cs2, l1t, l2t = _build_tables()
    ident32 = np.eye(128, dtype=np.float32)
    pred = np.ascontiguousarray(pred_corners, dtype=np.float32).reshape(B_TOTAL, 24)
    targ = np.ascontiguousarray(target_corners, dtype=np.float32).reshape(B_TOTAL, 24)

    in_maps = []
    for k in range(N_CORES):
        sl = slice(k * B_CORE, (k + 1) * B_CORE)
        hbuf = np.zeros((128, W_IN), dtype=np.float16)
        hbuf[0:24, C_DATA:C_DATA + B_CORE] = pred[sl].T.astype(np.float16)
        hbuf[24:48, C_DATA:C_DATA + B_CORE] = (-targ[sl].T).astype(np.float16)
        hbuf[0:48, C_A1:C_A1 + 128] = a1
        hbuf[0:48, C_A2:C_A2 + 64] = a2
        hbuf[0:128, C_CS1:C_CS1 + 64] = cs1
        hbuf[0:64, C_CS2:C_CS2 + 64] = cs2
        hbuf[0:64, C_L1:C_L1 + 224] = l1t
        hbuf[0:112, C_L2:C_L2 + 840] = l2t
        in_maps.append({"hbuf": hbuf, "ident32": ident32})
    return in_maps


def build_nc():
    nc = bacc.Bacc("TRN2", target_bir_lowering=False, debug=False)

    hbuf_d = nc.dram_tensor("hbuf", [128, W_IN], F16, kind="ExternalInput")
    id_d = nc.dram_tensor("ident32", [128, 128], F32, kind="ExternalInput")
    out_d = nc.dram_tensor("out", [B_CORE], F32, kind="ExternalOutput")

    AF = mybir.ActivationFunctionType
    ALU = mybir.AluOpType

    with tile.TileContext(nc) as tc:
        with (
            tc.tile_pool(name="consts", bufs=1) as cpool,
            tc.tile_pool(name="work", bufs=1) as wpool,
            tc.tile_pool(name="ps_x", bufs=2, space="PSUM") as psx,
            tc.tile_pool(name="ps_q", bufs=2, space="PSUM") as psq,
            tc.tile_pool(name="ps_w", bufs=1, space="PSUM") as psw,
            tc.tile_pool(name="ps_t", bufs=1, space="PSUM") as pst,
        ):
            IN = cpool.tile([128, W_IN], F16, tag="in")
            ID32 = cpool.tile([128, 128], F32, tag="id32")
            nc.sync.dma_start(IN[:, :], hbuf_d[:, :])
            nc.sync.dma_start(ID32[:, :], id_d[:, :])

            data = IN[0:48, C_DATA:C_DATA + B_CORE]
            a1 = IN[0:48, C_A1:C_A1 + 128]
            a2 = IN[0:48, C_A2:C_A2 + 64]
            cs1 = IN[0:128, C_CS1:C_CS1 + 64]
            cs2 = IN[0:64, C_CS2:C_CS2 + 64]
            l1a = IN[0:64, C_L1:C_L1 + 112]
            l1b = IN[0:64, C_L1 + 112:C_L1 + 224]
            l2a = IN[0:112, C_L2:C_L2 + 420]
            l2b = IN[0:112, C_L2 + 420:C_L2 + 840]

            # -- prologue during DMA wait: act table loads + PE p-state ramp
            warm = wpool.tile([128, 512], F16, tag="warm")
            nc.vector.memset(warm[:, :], 0.0)
            dummy = wpool.tile([1, 2], F16, tag="dummy")
            nc.scalar.activation(dummy[0:1, 0:1], warm[0:1, 0:1], AF.Square)
            nc.scalar.activation(dummy[0:1, 1:2], warm[0:1, 0:1], AF.Sqrt)
            psW = psw.tile([128, 512], F32, tag="w")
            for _ in range(N_WARMUP):
                nc.tensor.matmul(psW[:, :], warm[:, 0:128], warm[:, :],
                                 start=True, stop=True)

            # -- phase 1: diff -> square -> coord-sum -> sqrt => distT fp16
            psD1 = psx.tile([128, 512], F32, tag="x")
            nc.tensor.matmul(psD1[:, :], a1, data, start=True, stop=True)
            psD2 = psx.tile([128, 512], F32, tag="x")
            nc.tensor.matmul(psD2[0:64, :], a2, data, start=True, stop=True)

            sq1 = wpool.tile([128, 512], F16, tag="sq1")
            sq2 = wpool.tile([64, 512], F16, tag="sq2")
            nc.scalar.activation(sq1[:, :], psD1[:, :], AF.Square)
            nc.scalar.activation(sq2[:, :], psD2[0:64, :], AF.Square)

            psE = psx.tile([128, 512], F32, tag="x")
            nc.tensor.matmul(psE[0:64, :], cs1, sq1[:, :], start=True, stop=False)
            nc.tensor.matmul(psE[0:64, :], cs2, sq2[:, :], start=False, stop=True)

            distT = wpool.tile([64, 512], F16, tag="distT")
            nc.scalar.activation(distT[:, :], psE[0:64, :], AF.Sqrt)

            # -- L1: both orderings into one 2-bank tile, fused min reduce
            psL = psq.tile([128, 1024], F32, tag="q")
            nc.tensor.matmul(psL[0:112, 0:512], l1a, distT[:, :],
                             start=True, stop=True)
            nc.tensor.matmul(psL[0:112, 512:1024], l1b, distT[:, :],
                             start=True, stop=True)
            m_t = wpool.tile([112, 512], F16, tag="m")
            nc.vector.tensor_reduce(
                m_t[:, :],
                psL[0:112, :].rearrange("p (k j) -> p j k", k=2),
                axis=mybir.AxisListType.X, op=ALU.min)

            # -- L2 + L3 per 128-sample chunk
            minall = wpool.tile([128, 560], F16, tag="minall")
            msum = wpool.tile([128, 280], F16, tag="msum")
            loss = wpool.tile([128, 64], F32, tag="loss")
            nc.vector.memset(loss[:, :], 0.0)
            qb0 = wpool.tile([128, 840], F16, tag="qb0")
            qb1 = wpool.tile([128, 840], F16, tag="qb1")

            for c in range(N_CHUNKS):
                sl = slice(c * CHUNK, (c + 1) * CHUNK)
                psQ = psq.tile([128, 1024], F32, tag="q")
                nc.tensor.matmul(psQ[:, 0:420], m_t[:, sl], l2a,
                                 start=True, stop=True)
                nc.tensor.matmul(psQ[:, 512:932], m_t[:, sl], l2b,
                                 start=True, stop=True)

                psv = (psQ[:, :].rearrange("p (k j) -> p k j", k=2)[:, :, 0:420]
                       .rearrange("p k (t s) -> p k t s", s=6))
                mout = minall[:, c * 140:(c + 1) * 140]
                if c < 2:
                    qb = (qb0, qb1)[c]
                    nc.scalar.activation(
                        qb[:, :].rearrange("p (k x) -> p k x", k=2),
                        psQ[:, :].rearrange("p (k j) -> p k j", k=2)[:, :, 0:420],
                        AF.Copy)
                    nc.vector.tensor_reduce(
                        mout, qb[:, :].rearrange("p (k t s) -> p k t s", k=2, s=6),
                        axis=mybir.AxisListType.X, op=ALU.min)
                else:
                    nc.vector.tensor_reduce(
                        mout, psv, axis=mybir.AxisListType.X, op=ALU.min)

                # A+B add (Pool, SBUF-only) then min over 70 -> loss[:, c]
                nc.gpsimd.tensor_tensor(
                    msum[:, c * 70:(c + 1) * 70],
                    minall[:, c * 140:c * 140 + 70],
                    minall[:, c * 140 + 70:c * 140 + 140],
                    op=ALU.add)
                nc.vector.tensor_reduce(
                    loss[:, c:c + 1], msum[:, c * 70:(c + 1) * 70],
                    axis=mybir.AxisListType.X, op=ALU.min)

            # -- finale: fp32 transpose [128,64] -> [64,128], one 2KB DMA
            psT = pst.tile([64, 128], F32, tag="t")
            nc.tensor.transpose(psT[:, :], loss[:, :], ID32[:, :])
            outb = wpool.tile([4, 128], F32, tag="outb")
            nc.scalar.activation(outb[:, :], psT[0:4, :], AF.Copy)
            nc.sync.dma_start(
                out_d[:].rearrange("(c p) -> c p", p=128), outb[:, :])

    nc.compile()
    return nc


_NC = None


def _get_nc():
    global _NC
    if _NC is None:
        _NC = build_nc()
    return _NC


def kernel(pred_corners: np.ndarray, target_corners: np.ndarray) -> np.ndarray:
    from concourse.bass_utils import run_bass_kernel_spmd

    nc = _get_nc()
    in_maps = make_in_maps(pred_corners, target_corners)
    res = run_bass_kernel_spmd(nc, in_maps, core_ids=list(range(N_CORES)))
    return np.concatenate([res.results[k]["out"] for k in range(N_CORES)])
